# revision 1
# baseline (speedup 1.0000x reference)
"""Trainium2 Bass kernel for nn_Attention_13537736917778.

Full inputs -> full output. Sharding: 8 cores = 2 (batch) x 4 (head groups of 4).
Per-core: channel-major flash attention (S^T layout, keys on partitions).
Softmax denominators are produced replicated across 64 PSUM rows by ones-columns
in the PV stationary operand; normalization and RMS-norm reciprocals both run as
ACT ln/exp pairs (exp table set only), so no DVE reciprocal and no PE broadcast
matmuls. Out-projection partial sums are reduced on host.
"""
import sys
import numpy as np

sys.path.insert(0, "/opt/trn_rl_repo")

import ml_dtypes
import concourse.bass as bass
import concourse.mybir as mybir
from concourse import tile
from concourse.bass_utils import run_bass_kernel_spmd
from contextlib import ExitStack

bf16 = mybir.dt.bfloat16
f32 = mybir.dt.float32

B, N, C = 2, 2048, 1024
H, D = 16, 64
G = 4              # heads per core
NT = N             # tokens per core (one batch)
FT = 512
TI = NT // FT      # 4 i-tiles
KC = C // 128      # 8 input-channel chunks
JC = NT // 128     # 16 key chunks
OC = 3 * G * D // 128   # 6 qkv output chunks
EPS = 1e-6
SCALE = 1.0 / 8.0  # 1/sqrt(64)

_CACHE = {}


def _split_waits(nc, limit=1):
    """walrus CTRL has one hw wait slot; split multi-wait instructions into
    NOP chains carrying the extra waits."""
    counter = 0
    for fn in nc.m.functions:
        for bb in fn.blocks:
            new_insts = []
            for inst in bb.instructions:
                si = inst.sync_info
                if si is not None and si.on_wait and len(si.on_wait) > limit:
                    waits = list(si.on_wait)
                    head, tail = waits[:-limit], waits[-limit:]
                    for i in range(0, len(head), limit):
                        nop = mybir.InstNoOp(
                            name=f"I-waitsplit-{counter}", ins=[], outs=[]
                        )
                        counter += 1
                        nop.engine = inst.engine
                        nop.sync_info = mybir.SyncInfo(
                            on_wait=head[i : i + limit], on_update=[]
                        )
                        new_insts.append(nop)
                    inst.sync_info = mybir.SyncInfo(
                        on_wait=tail, on_update=list(si.on_update)
                    )
                new_insts.append(inst)
            bb.instructions[:] = new_insts
    return counter


def _build_nc():
    nc = bass.Bass()
    xT = nc.declare_dram_parameter("xT", [C, NT], bf16, isOutput=False)
    wqkvT = nc.declare_dram_parameter("wqkvT", [C, OC * 128], bf16, isOutput=False)
    bqkv = nc.declare_dram_parameter("bqkv", [128, OC], f32, isOutput=False)
    wrep = nc.declare_dram_parameter("wrep", [128, 2], f32, isOutput=False)
    iden = nc.declare_dram_parameter("iden", [128, 128], bf16, isOutput=False)
    woT = nc.declare_dram_parameter("woT", [2, 128, C], bf16, isOutput=False)
    y = nc.declare_dram_parameter("y", [NT, C], f32, isOutput=True)

    xT_r = xT.rearrange("(kc p) n -> kc p n", p=128)

    Exp = mybir.ActivationFunctionType.Exp
    Ln = mybir.ActivationFunctionType.Ln
    Ident = mybir.ActivationFunctionType.Identity
    CopyF = mybir.ActivationFunctionType.Copy
    MUL = mybir.AluOpType.mult

    with tile.TileContext(nc) as tc:
        with ExitStack() as ctx:
            perm = ctx.enter_context(tc.tile_pool(name="perm", bufs=1))
            iden_sb = perm.tile([128, 128], bf16, name="iden_sb", tag="iden_sb")
            nc.sync.dma_start(out=iden_sb[:], in_=iden[:])
            bias_sb = perm.tile([128, OC + 1], f32, name="bias_sb", tag="bias_sb")
            nc.sync.dma_start(out=bias_sb[:, 0:OC], in_=bqkv[:])
            nc.vector.memset(bias_sb[:, OC : OC + 1], EPS)
            wrep_sb = perm.tile([128, 2], f32, name="wrep_sb", tag="wrep_sb")
            nc.sync.dma_start(out=wrep_sb[:], in_=wrep[:])
            ones_mask = perm.tile([128, 128], bf16, name="ones_mask", tag="ones_mask")
            nc.vector.memset(ones_mask[:], 0.0)
            nc.vector.memset(ones_mask[0:64, 0:64], 1.0)
            nc.vector.memset(ones_mask[64:128, 64:128], 1.0)

            w_all = perm.tile([128, KC, OC * 128], bf16, name="w_all", tag="w_all")
            woT_sb = []
            for oo in range(2):
                t = perm.tile([128, C], bf16, name=f"woT_sb{oo}", tag=f"woT_sb{oo}")
                nc.sync.dma_start(out=t[:], in_=woT[oo])
                woT_sb.append(t)

            qk_sb = [
                perm.tile([128, NT], f32, name=f"qk_sb{i}", tag=f"qk_sb{i}")
                for i in range(4)
            ]
            v16_sb = [
                perm.tile([128, NT], bf16, name=f"v16_sb{i}", tag=f"v16_sb{i}")
                for i in range(2)
            ]
            qhat = [
                perm.tile([128, NT], bf16, name=f"qhat{i}", tag=f"qhat{i}")
                for i in range(2)
            ]
            khat = [
                perm.tile([128, NT], bf16, name=f"khat{i}", tag=f"khat{i}")
                for i in range(2)
            ]
            vtok = [
                perm.tile([128, JC, 256], bf16, name=f"vtok{i}", tag=f"vtok{i}")
                for i in range(2)
            ]
            onT = [
                perm.tile([128, NT], bf16, name=f"onT{i}", tag=f"onT{i}")
                for i in range(2)
            ]
            for c in range(2):
                nc.vector.memset(vtok[c][:, :, 64:128], 1.0)
                nc.vector.memset(vtok[c][:, :, 192:256], 1.0)

            sqpool = ctx.enter_context(tc.tile_pool(name="sqpool", bufs=2))
            t1pool = ctx.enter_context(tc.tile_pool(name="t1pool", bufs=1))
            rrpool = ctx.enter_context(tc.tile_pool(name="rrpool", bufs=2))
            ptpool = ctx.enter_context(tc.tile_pool(name="ptpool", bufs=6))
            dnpool = ctx.enter_context(tc.tile_pool(name="dnpool", bufs=2))
            ypool = ctx.enter_context(tc.tile_pool(name="ypool", bufs=2))
            spool = ctx.enter_context(tc.tile_pool(name="spool", bufs=2, space="PSUM"))
            pvpool = ctx.enter_context(
                tc.tile_pool(name="pvpool", bufs=1, space="PSUM")
            )

            cnt = [0]
            xt_all = perm.tile([128, KC, NT], bf16, name="xt_all", tag="xt_all")

            def emit_x_dma(kc):
                nc.sync.dma_start(out=xt_all[:, kc, :], in_=xT_r[kc])

            def emit_qkv_pair(itp, oc):
                # both i-tiles of the pair share each kc's weights (LDW dedupe)
                i0 = slice((2 * itp) * FT, (2 * itp + 1) * FT)
                i1 = slice((2 * itp + 1) * FT, (2 * itp + 2) * FT)
                cnt[0] += 1
                ps = spool.tile([128, 3 * FT], f32, name=f"qp{cnt[0]}", tag="s")
                for kc in range(KC):
                    w = w_all[:, kc, oc * 128 : (oc + 1) * 128]
                    nc.tensor.matmul(
                        ps[:, 0:FT], lhsT=w, rhs=xt_all[:, kc, i0],
                        start=(kc == 0), stop=(kc == KC - 1),
                    )
                    nc.tensor.matmul(
                        ps[:, FT : 2 * FT], lhsT=w, rhs=xt_all[:, kc, i1],
                        start=(kc == 0), stop=(kc == KC - 1),
                    )
                dst = qk_sb[oc] if oc < 4 else v16_sb[oc - 4]
                nc.vector.tensor_scalar_add(
                    dst[:, i0], ps[:, 0:FT], bias_sb[:, oc : oc + 1]
                )
                nc.vector.tensor_scalar_add(
                    dst[:, i1], ps[:, FT : 2 * FT], bias_sb[:, oc : oc + 1]
                )

            def emit_vtrans(c, jc):
                cnt[0] += 1
                pt = spool.tile([128, 3 * FT], bf16, name=f"vt{cnt[0]}", tag="s")
                nc.tensor.transpose(
                    pt[:, 0:128], v16_sb[c][:, jc * 128 : (jc + 1) * 128], iden_sb[:]
                )
                nc.vector.tensor_copy(vtok[c][:, jc, 0:64], pt[:, 0:64])
                nc.vector.tensor_copy(vtok[c][:, jc, 128:192], pt[:, 64:128])

            def emit_rms(qc, qk_i):
                src_t = qk_sb[qc] if qk_i == 0 else qk_sb[2 + qc]
                dst = qhat[qc] if qk_i == 0 else khat[qc]
                sq = sqpool.tile([128, NT], bf16, name=f"sq{qc}_{qk_i}", tag="sq")
                nc.vector.tensor_mul(sq[:], src_t[:], src_t[:])
                rr = rrpool.tile([128, NT], f32, name=f"rr{qc}_{qk_i}", tag="rr")
                for half in range(2):
                    hs = slice(half * 2 * FT, (half + 1) * 2 * FT)
                    cnt[0] += 1
                    ms = pvpool.tile([128, 2 * FT], f32, name=f"ms{cnt[0]}", tag="pv")
                    for t in range(2):
                        tsl = slice(t * FT, (t + 1) * FT)
                        gsl = slice((half * 2 + t) * FT, (half * 2 + t + 1) * FT)
                        nc.tensor.matmul(
                            ms[:, tsl],
                            lhsT=ones_mask[:],
                            rhs=sq[:, gsl],
                            start=True,
                            stop=True,
                        )
                    t1 = t1pool.tile(
                        [128, 2 * FT], f32, name=f"t1_{qc}_{qk_i}_{half}", tag="t1"
                    )
                    nc.scalar.activation(
                        t1[:], ms[:], Ln, scale=1.0 / D, bias=bias_sb[:, OC : OC + 1]
                    )
                    nc.scalar.activation(rr[:, hs], t1[:], Exp, scale=-0.5)
                nc.vector.scalar_tensor_tensor(
                    dst[:], src_t[:], wrep_sb[:, qk_i : qk_i + 1], rr[:], MUL, MUL
                )

            def emit_attention(qc, it):
                # seq of 32 (head, jc) S-blocks, 3 per PSUM tile; PV emission is
                # skewed 2 tiles behind exp so PE never queue-stalls on the
                # normalize chain of the previous i-tile
                isl = slice(it * FT, (it + 1) * FT)
                pv2 = pvpool.tile([128, 2 * FT], f32, name=f"pv2_{qc}_{it}", tag="pv")
                NSEQ = 2 * JC
                PER = 3
                s3 = None
                pending = []  # (pt_tile, [seq indices]) awaiting PV emission
                tiles_done = 0

                def emit_pv(pt3, seqs):
                    for sb in seqs:
                        hb = sb % 2
                        jb = sb // 2
                        bsl = slice((sb % PER) * FT, (sb % PER + 1) * FT)
                        nc.tensor.matmul(
                            pv2[:, hb * FT : (hb + 1) * FT],
                            lhsT=vtok[qc][:, jb, hb * 128 : (hb + 1) * 128],
                            rhs=pt3[:, bsl],
                            start=(sb == hb),
                            stop=(sb >= NSEQ - 2),
                        )

                tiles = []  # list of (start_seq, nseqs)
                sidx = 0
                while sidx < NSEQ:
                    n = min(PER, NSEQ - sidx)
                    tiles.append((sidx, n))
                    sidx += n

                def emit_s_tile(t0, n):
                    s3 = spool.tile(
                        [128, PER * FT], f32, name=f"s3_{qc}_{it}_{t0}", tag="s"
                    )
                    for k in range(n):
                        s = t0 + k
                        head = s % 2
                        jc = s // 2
                        nc.tensor.matmul(
                            s3[:, k * FT : (k + 1) * FT],
                            lhsT=khat[qc][head * 64 : (head + 1) * 64,
                                          jc * 128 : (jc + 1) * 128],
                            rhs=qhat[qc][head * 64 : (head + 1) * 64, isl],
                            start=True,
                            stop=True,
                        )
                    return s3

                def emit_exp(s3, t0, n):
                    pt3 = ptpool.tile(
                        [128, PER * FT], bf16, name=f"pt{qc}_{it}_{t0}", tag="pt"
                    )
                    nc.scalar.activation(
                        pt3[:, 0 : n * FT], s3[:, 0 : n * FT], Exp, scale=SCALE
                    )
                    pending.append((pt3, list(range(t0, t0 + n))))

                ti = 0
                while ti < len(tiles):
                    grp = tiles[ti : ti + 2]
                    ti += len(grp)
                    s3s = [emit_s_tile(t0, n) for t0, n in grp]
                    for s3g, (t0, n) in zip(s3s, grp):
                        emit_exp(s3g, t0, n)
                    while len(pending) > 2:
                        emit_pv(*pending.pop(0))
                for args in pending:
                    emit_pv(*args)
                # normalize: O = PV * exp(-ln(denom)); denom replicated rows 64:128
                td = dnpool.tile([64, 2 * FT], f32, name=f"td{qc}_{it}", tag="td")
                nc.scalar.activation(td[:], pv2[64:128, :], Ln)
                bcr = dnpool.tile([64, 2 * FT], f32, name=f"bcr{qc}_{it}", tag="bcr")
                nc.scalar.activation(bcr[:], td[:], Exp, scale=-1.0)
                nc.vector.tensor_mul(onT[qc][0:64, isl], pv2[0:64, 0:FT], bcr[:, 0:FT])
                nc.vector.tensor_mul(
                    onT[qc][64:128, isl], pv2[0:64, FT : 2 * FT], bcr[:, FT : 2 * FT]
                )

            def emit_outproj(ic):
                csl = slice(ic * 128, (ic + 1) * 128)
                cnt[0] += 1
                p01 = spool.tile([128, 3 * FT], f32, name=f"p01_{ic}", tag="s")
                for oo in range(2):
                    nc.tensor.matmul(
                        p01[:, 0:FT],
                        lhsT=onT[oo][:, csl],
                        rhs=woT_sb[oo][:, 0:FT],
                        start=(oo == 0),
                        stop=(oo == 1),
                    )
                    nc.tensor.matmul(
                        p01[:, FT : 2 * FT],
                        lhsT=onT[oo][:, csl],
                        rhs=woT_sb[oo][:, FT : 2 * FT],
                        start=(oo == 0),
                        stop=(oo == 1),
                    )
                yt = ypool.tile([128, C], f32, name=f"yt{ic}", tag="yt")
                nc.vector.tensor_copy(yt[:], p01[:, 0 : 2 * FT])
                nc.sync.dma_start(out=y[csl, :], in_=yt[:])

            # ---------------- emission schedule ----------------
            wqkvT_r2 = wqkvT.rearrange("(kc p) o -> kc p o", p=128)
            for kc in range(KC):
                nc.sync.dma_start(out=w_all[:, kc, :], in_=wqkvT_r2[kc])
                emit_x_dma(kc)
            for itp in range(2):
                emit_qkv_pair(itp, 0)
            emit_rms(0, 0)
            for itp in range(2):
                emit_qkv_pair(itp, 2)
            emit_rms(0, 1)
            for itp in range(2):
                emit_qkv_pair(itp, 4)
                for jc in range(itp * 8, itp * 8 + 8):
                    emit_vtrans(0, jc)
            for it in range(TI):
                emit_attention(0, it)
            for itp in range(2):
                emit_qkv_pair(itp, 1)
            emit_rms(1, 0)
            for itp in range(2):
                emit_qkv_pair(itp, 3)
            emit_rms(1, 1)
            for itp in range(2):
                emit_qkv_pair(itp, 5)
                for jc in range(itp * 8, itp * 8 + 8):
                    emit_vtrans(1, jc)
            for it in range(TI):
                emit_attention(1, it)
                for ic in range(it * 4, it * 4 + 4):
                    emit_outproj(ic)

    _split_waits(nc, limit=1)
    _dedupe_ldweights(nc)
    return nc


def _dedupe_ldweights(nc):
    """Drop an InstLdweights identical to the previous one on the PE stream
    (only Matmult/NoOp between) -- the PE keeps the loaded weights."""
    import concourse.mybir as mybir
    pe = mybir.EngineType.PE
    ndel = 0
    for fn in nc.m.functions:
        for bb in fn.blocks:
            new = []
            last = None
            for inst in bb.instructions:
                tn = type(inst).__name__
                if getattr(inst, "engine", None) == pe:
                    if tn == "InstLdweights":
                        key = str(inst.ins[0])
                        if last == key and not (
                            inst.sync_info and inst.sync_info.on_wait
                        ):
                            ndel += 1
                            continue
                        last = key
                    elif tn not in ("InstMatmult", "InstNoOp"):
                        last = None
                new.append(inst)
            bb.instructions[:] = new
    return ndel


def _prep_inputs(x, Wq, bq, Wk, bk, Wv, bv, q_norm_w, k_norm_w, Wo, bo):
    bf = ml_dtypes.bfloat16
    x = np.asarray(x, dtype=np.float32)
    Wfull = np.concatenate(
        [np.asarray(Wq), np.asarray(Wk), np.asarray(Wv)], axis=0
    ).astype(np.float32)
    bfull = np.concatenate(
        [np.asarray(bq), np.asarray(bk), np.asarray(bv)], axis=0
    ).astype(np.float32)
    Wo = np.asarray(Wo, dtype=np.float32)
    q_norm_w = np.asarray(q_norm_w, dtype=np.float32)
    k_norm_w = np.asarray(k_norm_w, dtype=np.float32)

    xT_b = [np.ascontiguousarray(x[b].T).astype(bf) for b in range(B)]
    IDEN = np.eye(128, dtype=np.float32).astype(bf)
    wrep = np.stack(
        [np.tile(q_norm_w, 2), np.tile(k_norm_w, 2)], axis=1
    ).astype(np.float32)

    in_maps = []
    for core in range(8):
        b = core // 4
        hg = core % 4
        heads = [hg * 4 + i for i in range(G)]
        q_rows = np.concatenate([Wfull[192 * h : 192 * h + 64] for h in heads], axis=0)
        k_rows = np.concatenate(
            [Wfull[192 * h + 64 : 192 * h + 128] for h in heads], axis=0
        )
        v_rows = np.concatenate(
            [Wfull[192 * h + 128 : 192 * h + 192] for h in heads], axis=0
        )
        W_shard = np.concatenate([q_rows, k_rows, v_rows], axis=0)  # [768, 1024]
        bq_rows = np.concatenate([bfull[192 * h : 192 * h + 64] for h in heads])
        bk_rows = np.concatenate([bfull[192 * h + 64 : 192 * h + 128] for h in heads])
        bv_rows = np.concatenate([bfull[192 * h + 128 : 192 * h + 192] for h in heads])
        b_shard = np.concatenate([bq_rows, bk_rows, bv_rows])  # [768]
        cols = np.concatenate([np.arange(64 * h, 64 * h + 64) for h in heads])
        WoT_shard = np.ascontiguousarray(Wo[:, cols].T)  # [256, 1024]


        in_maps.append(
            {
                "xT": xT_b[b],
                "wqkvT": np.ascontiguousarray(W_shard.T).astype(bf),
                "bqkv": np.ascontiguousarray(b_shard.reshape(OC, 128).T).astype(
                    np.float32
                ),
                "wrep": wrep,
                "iden": IDEN,
                "woT": WoT_shard.reshape(2, 128, C).astype(bf),
            }
        )
    return in_maps


def kernel(**inputs):
    if "nc" not in _CACHE:
        _CACHE["nc"] = _build_nc()
    nc = _CACHE["nc"]
    in_maps = _prep_inputs(**inputs)
    res = run_bass_kernel_spmd(nc, in_maps, list(range(8)))
    bo = np.asarray(inputs["bo"], dtype=np.float32)
    y = np.zeros((B, N, C), dtype=np.float32)
    for core in range(8):
        y[core // 4] += res.results[core]["y"]
    y += bo[None, None, :]
    return y

